# revision 9
# baseline (speedup 1.0000x reference)
"""Trainium2 Bass kernel: block 8x8 2D-DCT + channel-pack + 8x nearest upsample.

Computes, for input x (8, 3, 256, 256) f32:
  out[b, 64c+8a+d, 8i+r, 8j+q] = sum_{m,n} D[a,m] x[b,c,8i+m,8j+n] D[d,n]
i.e. the reference nn_DCT2D: per-8x8-block orthonormal DCT-II, 64 coeffs packed
into channels, then 8x8 nearest-neighbor upsample back to (256, 256).

Strategy (pure data-parallel over batch, one core per batch element):
  - Step 1 (TensorE): A'^T = X^T @ M', where M' is the block-diagonal
    (I_32 (x) D^T) with columns permuted to a' = 32a + i. Two [128,256]
    matmul pairs accumulating over the 256-row contraction.
  - Step 2 (TensorE): out2 = A' @ R, where the constant R both applies the
    second DCT factor (over n) and replicates each output 8x along W
    (columns (d, ww) with ww = 8j+q repeated over q). Result rows are
    a' = 32a+i, columns (d, ww) -- i.e. W-upsampled coefficient rows.
  - DMA out: the H-direction 8x replication is done by the output DMA: the
    same SBUF rows are written to 8 row-phases r of the output image.

Everything is f32; matmul accumulation in PSUM f32.
"""

import numpy as np

import concourse.bacc as bacc
import concourse.mybir as mybir
from concourse.tile import TileContext
from concourse.bass_utils import run_bass_kernel_spmd

N_CORES = 8
B, C, H, W = 8, 3, 256, 256
BS = 8          # DCT block size
HB = H // BS    # 32 blocks per side
F32 = mybir.dt.float32


def _dct_matrix() -> np.ndarray:
    n = np.arange(BS, dtype=np.float64)
    k = n[:, None]
    D = np.cos(np.pi * (2.0 * n[None, :] + 1.0) * k / (2.0 * BS))
    scale = np.full((BS,), np.sqrt(2.0 / BS))
    scale[0] = np.sqrt(1.0 / BS)
    return (D * scale[:, None]).astype(np.float32)


def _build_consts() -> tuple[np.ndarray, np.ndarray]:
    D = _dct_matrix()
    # M' [256, 256]: M'[k, 32a+i] = D[a, k%8] if k//8 == i else 0
    Mp = np.zeros((256, 256), np.float32)
    for k in range(256):
        for a in range(8):
            Mp[k, 32 * a + (k // 8)] = D[a, k % 8]
    M = np.stack([Mp[:128], Mp[128:]])  # [2, 128, 256]

    # R [2, 4, 128, 512]: for (kh, dp): cols q: d = 2dp + q//256, ww = q%256,
    # j = ww//8; nonzero iff j in [16kh, 16kh+16) and k'//8 == j - 16kh;
    # value D[d, k'%8].
    R = np.zeros((2, 4, 128, 512), np.float32)
    kp = np.arange(128)
    for kh in range(2):
        for dp in range(4):
            for q in range(512):
                d = 2 * dp + q // 256
                j = (q % 256) // 8
                if 16 * kh <= j < 16 * kh + 16:
                    rows = kp[kp // 8 == j - 16 * kh]
                    R[kh, dp, rows, q] = D[d, rows % 8]
    return M, R


def _build_module():
    nc = bacc.Bacc("TRN2", target_bir_lowering=False, debug=False,
                   enable_asserts=False)

    x_t = nc.dram_tensor("x", [C, H, W], F32, kind="ExternalInput")
    m_t = nc.dram_tensor("m", [2, 128, 256], F32, kind="ExternalInput")
    r_t = nc.dram_tensor("r", [2, 4, 128, 512], F32, kind="ExternalInput")
    out_t = nc.dram_tensor("out", [C * 64, H, W], F32, kind="ExternalOutput")

    with TileContext(nc) as tc:
        with (
            tc.tile_pool(name="consts", bufs=1) as cpool,
            tc.tile_pool(name="xp", bufs=4) as xpool,
            tc.tile_pool(name="atp", bufs=4) as atpool,
            tc.tile_pool(name="outp", bufs=2) as opool,
            tc.tile_pool(name="psa", bufs=2, space="PSUM") as psa_pool,
            tc.tile_pool(name="ps2", bufs=6, space="PSUM") as ps2_pool,
        ):
            m_tiles = [cpool.tile_from(m_t[kt], name=f"m{kt}")
                       for kt in range(2)]
            r_tiles = [[cpool.tile_from(r_t[kh, dp], name=f"r{kh}{dp}")
                        for dp in range(4)]
                       for kh in range(2)]

            for c in range(C):
                # load image c as two [128, 256] row tiles
                xt = []
                for kt in range(2):
                    tile = xpool.tile([128, 256], F32, tag="x")
                    nc.sync.dma_start(out=tile[:, :],
                                      in_=x_t[c, kt * 128:(kt + 1) * 128, :])
                    xt.append(tile)

                # step 1: A'^T[n_img, a'] in two row-halves kh
                at = []
                for kh in range(2):
                    ps_a = psa_pool.tile([128, 256], F32, tag="psa")
                    for kt in range(2):
                        nc.tensor.matmul(
                            ps_a[:, :],
                            lhsT=xt[kt][:, kh * 128:(kh + 1) * 128],
                            rhs=m_tiles[kt][:, :],
                            start=(kt == 0), stop=(kt == 1),
                        )
                    a_sb = atpool.tile([128, 256], F32, tag="at")
                    nc.any.tensor_copy(out=a_sb[:, :], in_=ps_a[:, :])
                    at.append(a_sb)

                # step 2: out2_t[a'-128t, d*2048 + r*256 + ww], where the
                # W-upsampled coefficient row for (a', d) is duplicated
                # r=8x so each DMA descriptor covers the full 8-row output
                # block (8 KB contiguous in HBM).
                for t in range(2):
                    o2 = opool.tile([128, 16384], F32, tag="o2")
                    for dp in range(4):
                        ps2 = ps2_pool.tile([128, 512], F32, tag="ps2")
                        for kh in range(2):
                            nc.tensor.matmul(
                                ps2[:, :],
                                lhsT=at[kh][:, t * 128:(t + 1) * 128],
                                rhs=r_tiles[kh][dp][:, :],
                                start=(kh == 0), stop=(kh == 1),
                            )
                        # copy + 8x duplicate: out cols d*2048+r*256+ww.
                        # Split between DVE and ACT for throughput.
                        for dd in range(2):
                            d = 2 * dp + dd
                            src_bc = ps2[:, dd * 256:(dd + 1) * 256]
                            src_bc = src_bc[:, None, :].to_broadcast(
                                [128, 8, 256])
                            dst = o2[:, d * 2048:(d + 1) * 2048].rearrange(
                                "p (rep w) -> p rep w", rep=8)
                            if (dp + dd) % 2 == 0:
                                nc.vector.tensor_copy(out=dst, in_=src_bc)
                            else:
                                nc.scalar.copy(out=dst, in_=src_bc)

                    # DMA out: one 2 MB DMA per a-group writes the full
                    # 8-channel, fully H/W-upsampled block; descriptors are
                    # the 8 KB contiguous 8-row blocks. Split across both
                    # HWDGE rings (sync + scalar).
                    for asub in range(4):
                        a = t * 4 + asub
                        ch0 = 64 * c + 8 * a
                        src = o2[32 * asub:32 * asub + 32, :].rearrange(
                            "p (d f) -> p d f", d=8)
                        dst = out_t[ch0:ch0 + 8].rearrange(
                            "d (i f) w -> d i (f w)", f=8).transpose([1, 0, 2])
                        eng = nc.sync if a % 2 == 0 else nc.scalar
                        eng.dma_start(out=dst, in_=src)

    nc.compile()
    return nc


_CACHE: dict = {}


def _get_module():
    if "nc" not in _CACHE:
        _CACHE["nc"] = _build_module()
        _CACHE["consts"] = _build_consts()
    return _CACHE["nc"], _CACHE["consts"]


def kernel(x: np.ndarray) -> np.ndarray:
    x = np.ascontiguousarray(np.asarray(x, dtype=np.float32))
    assert x.shape == (B, C, H, W), x.shape

    nc, (M, R) = _get_module()
    in_maps = [{"x": x[b], "m": M, "r": R} for b in range(N_CORES)]
    res = run_bass_kernel_spmd(nc, in_maps, core_ids=list(range(N_CORES)))
    out = np.stack([res.results[b]["out"] for b in range(N_CORES)], axis=0)
    return out
